# revision 15
# baseline (speedup 1.0000x reference)
"""Bass/Trainium2 kernel for CausalSelfAttention (B=8, T=1024, C=768, H=12).

Sharding: data-parallel over batch. 8 cores, one batch element per core.
No collectives. Each core runs an identical SPMD program on its own slice.

Schedule: QK/V projection chains are interleaved as PE filler inside the
attention kt-loops so the tensor engine never idles (keeps the PE DVFS
p-state at max clock). PV matmuls lag the S matmuls by one kt step so the
Exp on the scalar engine is off the PE critical path.

Per-core layouts (host-prepared):
  xT   [768, 1024] bf16   x[b].T
  wqk  [768, 1536] bf16   W_attn[:, :1536], Q columns pre-scaled by 1/sqrt(64)
  wv   [768, 768]  bf16   W_attn[:, 1536:]
  wp   [768, 768]  bf16   W_proj
  bqk  [128, 12]  f32     b_attn[:1536] per-tile columns (Q part pre-scaled)
  bv1  [1, 768]   f32     b_attn[1536:]  (broadcast on-chip)
  bp1  [1, 768]   f32     b_proj         (broadcast on-chip)
  qm   [128, 8]   f32     query_mask as per-partition columns per q-tile
  dm   [128, 8, 128] bf16 diagonal-block multiplicative masks, transposed
Output: y [1024, 768] f32 per core.
"""

import sys

if "/opt/trn_rl_repo" not in sys.path:
    sys.path.insert(0, "/opt/trn_rl_repo")

import numpy as np
import ml_dtypes

import concourse.bass as bass
import concourse.bacc as bacc
import concourse.mybir as mybir
import concourse.tile as tile
from concourse.bass import ts, ds

BF16 = mybir.dt.bfloat16
F32 = mybir.dt.float32
AF = mybir.ActivationFunctionType
ALU = mybir.AluOpType
BF16NP = ml_dtypes.bfloat16

T, C, H, HD = 1024, 768, 12, 64
NCORES = 8

_CACHE = {}


def build_program():
    """Build the single-core SPMD Bass program."""
    nc = bacc.Bacc("TRN2", target_bir_lowering=False, debug=False)

    xT_d = nc.dram_tensor("xT", [128, 6, T], BF16, kind="ExternalInput")
    wqk_d = nc.dram_tensor("wqk", [128, 6, 2 * C], BF16, kind="ExternalInput")
    wv_d = nc.dram_tensor("wv", [128, 6, C], BF16, kind="ExternalInput")
    wp_d = nc.dram_tensor("wp", [128, 6, C], BF16, kind="ExternalInput")
    bqk_d = nc.dram_tensor("bqk", [128, 12], F32, kind="ExternalInput")
    bv1_d = nc.dram_tensor("bv1", [1, C], F32, kind="ExternalInput")
    bp1_d = nc.dram_tensor("bp1", [1, C], F32, kind="ExternalInput")
    qm_d = nc.dram_tensor("qm", [128, 8], F32, kind="ExternalInput")
    dm_d = nc.dram_tensor("dm", [128, 8, 128], BF16, kind="ExternalInput")
    y_d = nc.dram_tensor("y", [T, C], F32, kind="ExternalOutput")

    with tile.TileContext(nc) as tc:
        with (
            tc.tile_pool(name="const", bufs=1) as cp,
            tc.tile_pool(name="ptp", bufs=10) as ptp,
            tc.tile_pool(name="recp", bufs=3) as recp,
            tc.tile_pool(name="bcp", bufs=3) as bcp,
            tc.tile_pool(name="otxp", bufs=3) as otxp,
            tc.tile_pool(name="ysb", bufs=3) as ysbp,
            tc.tile_pool(name="ps_a", bufs=5, space="PSUM") as ps_a,
            tc.tile_pool(name="ps_o", bufs=2, space="PSUM") as ps_o,
            tc.tile_pool(name="ps_bc", bufs=1, space="PSUM") as ps_bc,
        ):
            # ---------------- persistent SBUF tensors ----------------
            xT_sb = cp.tile([128, 6, T], BF16, name="xT_sb")
            wqk_sb = cp.tile([128, 6, 2 * C], BF16, name="wqk_sb")
            wv_sb = cp.tile([128, 6, C], BF16, name="wv_sb")
            wp_sb = cp.tile([128, 6, C], BF16, name="wp_sb")
            bqk_sb = cp.tile([128, 12], F32, name="bqk_sb")
            bv_sb = cp.tile([128, C], F32, name="bv_sb")
            bp_sb = cp.tile([128, C], F32, name="bp_sb")
            bv1_sb = cp.tile([1, C], F32, name="bv1_sb")
            bp1_sb = cp.tile([1, C], F32, name="bp1_sb")
            qm_sb = cp.tile([128, 8], F32, name="qm_sb")
            dm_sb = cp.tile([128, 8, 128], BF16, name="dm_sb")
            ones_bf = cp.tile([128, 64], BF16, name="ones_bf")
            qk_sb = [cp.tile([128, T], BF16, name=f"qk{m}") for m in range(12)]
            v_sb = [cp.tile([128, 12 * 65], BF16, name=f"v{t}") for t in range(8)]
            ot_sb = cp.tile([128, 6, T], BF16, name="ot_sb")

            # ---------------- loads (priority ordered) ----------------
            # DRAM tensors are host-shuffled to the SBUF layout [128, 6, X]
            # so full-tensor DMAs use long contiguous lines (full bandwidth).
            # Critical-path loads issue first; non-critical bulk issues are
            # held back behind a gpsimd op that depends on the first QK
            # chain, so they can't steal bandwidth from the ramp.
            nc.sync.dma_start(xT_sb[:, :, 0:512], xT_d[:, :, 0:512])
            nc.scalar.dma_start(wqk_sb[:, :, 0:384], wqk_d[:, :, 0:384])
            nc.scalar.dma_start(wqk_sb[:, :, 768:1152], wqk_d[:, :, 768:1152])
            nc.sync.dma_start(bqk_sb[:], bqk_d[:, :])
            nc.gpsimd.dma_start(wv_sb[:, :, :], wv_d[:, :, :])
            nc.sync.dma_start(xT_sb[:, :, 512:1024], xT_d[:, :, 512:1024])
            nc.scalar.dma_start(qm_sb[:], qm_d[:, :])
            nc.sync.dma_start(dm_sb[:], dm_d[:, :, :])
            nc.scalar.dma_start(bv1_sb[:], bv1_d[:, :])
            nc.sync.dma_start(bp1_sb[:], bp1_d[:, :])
            # warm the Exp activation table during the DMA ramp
            wrm = cp.tile([1, 16], F32, name="wrm")
            wrm2 = cp.tile([1, 16], F32, name="wrm2")
            nc.gpsimd.memset(wrm[:], 0.0)
            nc.scalar.activation(wrm2[:], wrm[:], AF.Exp)
            # on-chip broadcasts + constants
            nc.gpsimd.partition_broadcast(bv_sb[:], bv1_sb[:])
            nc.gpsimd.partition_broadcast(bp_sb[:], bp1_sb[:])
            nc.gpsimd.memset(ones_bf[:], 1.0)
            # ones columns interleaved into V (produce softmax sums during PV)
            for t in range(8):
                nc.gpsimd.memset(
                    v_sb[t].rearrange("p (h d) -> p h d", d=65)[:, :, 64:65], 1.0
                )
            # stall: delay the bulk issues until the first QK chain lands
            stall_sb = cp.tile([1, 4], BF16, name="stall_sb")
            nc.gpsimd.tensor_copy(stall_sb[:], qk_sb[0][0:1, 0:4])
            # bulk (needed later): rest of wqk, then wp
            nc.gpsimd.dma_start(wqk_sb[:, :, 384:768], wqk_d[:, :, 384:768])
            nc.gpsimd.dma_start(wqk_sb[:, :, 1152:1536], wqk_d[:, :, 1152:1536])
            nc.gpsimd.dma_start(wp_sb[:, :, :], wp_d[:, :, :])

            # ---------------- projection chains (used inline + as filler) ----------------
            def qk_chain(m, j):
                ps = ps_a.tile([128, 512], F32, name="ps", tag="a")
                for k in range(6):
                    nc.tensor.matmul(
                        ps[:],
                        wqk_sb[:, k, ts(m, 128)],
                        xT_sb[:, k, ts(j, 512)],
                        start=(k == 0),
                        stop=(k == 5),
                    )
                nc.vector.tensor_scalar(
                    qk_sb[m][:, ts(j, 512)],
                    ps[:],
                    bqk_sb[:, m : m + 1],
                    None,
                    op0=ALU.add,
                )

            def v_chain(t, half):
                c0, cw = (0, 512) if half == 0 else (512, 256)
                psv = ps_a.tile([128, 512], F32, name="psv", tag="a")
                for k in range(6):
                    nc.tensor.matmul(
                        psv[:, :cw],
                        xT_sb[:, k, ts(t, 128)],
                        wv_sb[:, k, ds(c0, cw)],
                        start=(k == 0),
                        stop=(k == 5),
                    )
                nh, h0 = cw // 64, c0 // 64
                nc.vector.tensor_add(
                    v_sb[t].rearrange("p (h d) -> p h d", d=65)[
                        :, h0 : h0 + nh, 0:64
                    ],
                    psv[:, :cw].rearrange("p (h d) -> p h d", d=64),
                    bv_sb[:, ds(c0, cw)].rearrange("p (h d) -> p h d", d=64),
                )

            def F(m, j):
                return lambda: qk_chain(m, j)

            def V(t, half):
                return lambda: v_chain(t, half)

            # ---------------- attention group: one head-pair, one query half ----------------
            def attention_group(pr, sbi, fills):
                hs = (2 * pr, 2 * pr + 1)
                q0 = 512 * sbi
                nkt = 4 + 4 * sbi
                psO = {
                    h: ps_o.tile([65, 512], F32, name="op", tag="op") for h in hs
                }
                ptts = {}

                def dopv(kt):
                    dc = max(0, kt * 128 - q0)
                    w = 512 - dc
                    for h in hs:
                        nc.tensor.matmul(
                            psO[h][:, ds(dc, w)],
                            v_sb[kt][:, h * 65 : h * 65 + 65],
                            ptts.pop((h, kt))[:, ds(dc, w)],
                            start=(kt == 0),
                            stop=(kt == nkt - 1),
                            skip_group_check=True,
                        )

                nf = len(fills)
                fi = 0
                for kt in range(nkt):
                    dc = max(0, kt * 128 - q0)
                    w = 512 - dc
                    for h in hs:
                        qp = (h % 2) * 64
                        sp = ps_a.tile([128, 512], F32, name="sp", tag="a")
                        nc.tensor.matmul(
                            sp[:, ds(dc, w)],
                            qk_sb[6 + pr][qp : qp + 64, ts(kt, 128)],
                            qk_sb[pr][qp : qp + 64, ds(q0 + dc, w)],
                            start=True,
                            stop=True,
                        )
                        ptt = ptp.tile([128, 512], BF16, name="ptt", tag="ptt")
                        nc.scalar.activation(
                            ptt[:, ds(dc, w)], sp[:, ds(dc, w)], AF.Exp
                        )
                        if kt * 128 >= q0:
                            nc.gpsimd.tensor_mul(
                                ptt[:, ds(dc, 128)],
                                ptt[:, ds(dc, 128)],
                                dm_sb[:, kt, :],
                            )
                        ptts[(h, kt)] = ptt
                    # pace the filler chains evenly across kt steps
                    tgt = ((kt + 1) * nf + nkt - 1) // nkt
                    while fi < tgt:
                        fills[fi]()
                        fi += 1
                    if kt >= 1:
                        dopv(kt - 1)
                dopv(nkt - 1)

                # normalize: OT = psO[0:64] / sum  (sum = psO row 64)
                # sums -> bf16 sbuf -> PE broadcast [64,512] -> approx recip
                for h in hs:
                    sums = recp.tile([65, 512], BF16, name="sums", tag="sums")
                    nc.vector.tensor_copy(sums[64:65, :], psO[h][64:65, :])
                    bc = ps_bc.tile([64, 512], F32, name="bc", tag="bc")
                    nc.tensor.matmul(
                        bc[:],
                        ones_bf[64:65, 0:64],
                        sums[64:65, :],
                        start=True,
                        stop=True,
                    )
                    bcs = bcp.tile([64, 512], F32, name="bcs", tag="bcs")
                    nc.vector.reciprocal_approx_fast(bcs[:], bc[:])
                    if h % 2 == 0:
                        nc.vector.tensor_mul(
                            ot_sb[0:64, pr, ds(q0, 512)],
                            psO[h][0:64, :],
                            bcs[:],
                        )
                    else:
                        otx = otxp.tile([64, 512], BF16, name="otx", tag="otx")
                        nc.vector.tensor_mul(otx[:], psO[h][0:64, :], bcs[:])
                        nc.sync.dma_start(
                            ot_sb[64:128, pr, ds(q0, 512)], otx[:]
                        )

            # ---------------- main schedule ----------------
            qk_chain(0, 0)
            qk_chain(6, 0)
            v_chain(0, 0)
            v_chain(0, 1)
            attention_group(
                0, 0,
                [F(0, 1), V(1, 0), V(1, 1), F(6, 1),
                 V(2, 0), V(2, 1), V(3, 0), V(3, 1)],
            )
            attention_group(
                0, 1,
                [V(4, 0), V(4, 1), V(5, 0), V(5, 1),
                 V(6, 0), V(6, 1), V(7, 0), V(7, 1),
                 F(1, 0), F(7, 0), F(1, 1), F(7, 1)],
            )
            for pr in (1, 2, 3):
                attention_group(pr, 0, [F(pr + 1, 0), F(pr + 7, 0)])
                attention_group(pr, 1, [F(pr + 1, 1), F(pr + 7, 1)])
            attention_group(4, 0, [F(5, 0), F(11, 0)])
            attention_group(4, 1, [F(5, 1)])
            attention_group(5, 0, [F(11, 1)])
            attention_group(5, 1, [])

            # ---------------- phase E: y = OT.T @ W_proj * qm + bp ----------------
            for qt in range(8):
                ysb = ysbp.tile([128, C], F32, name="ysb", tag="ysb")
                for c0, cw in ((0, 512), (512, 256)):
                    psy = ps_a.tile([128, 512], F32, name="psy", tag="a")
                    for k in range(6):
                        nc.tensor.matmul(
                            psy[:, :cw],
                            ot_sb[:, k, ts(qt, 128)],
                            wp_sb[:, k, ds(c0, cw)],
                            start=(k == 0),
                            stop=(k == 5),
                        )
                    nc.vector.scalar_tensor_tensor(
                        out=ysb[:, ds(c0, cw)],
                        in0=psy[:, :cw],
                        scalar=qm_sb[:, qt : qt + 1],
                        in1=bp_sb[:, ds(c0, cw)],
                        op0=ALU.mult,
                        op1=ALU.add,
                    )
                nc.sync.dma_start(y_d[ts(qt, 128), :], ysb[:])

    nc.compile()
    return nc


def _get_nc():
    if "nc" not in _CACHE:
        _CACHE["nc"] = build_program()
    return _CACHE["nc"]


def prep_core_inputs(x, mask, query_mask, W_attn, b_attn, W_proj, b_proj):
    """Host-side prep. Returns a list of per-core input dicts."""
    scale = 1.0 / np.sqrt(HD)
    W_s = np.asarray(W_attn, np.float32).copy()
    W_s[:, :C] *= scale
    b_s = np.asarray(b_attn, np.float32).copy()
    b_s[:C] *= scale

    def shuf(w):
        # [768, X] -> [128, 6, X]: partition-major layout matching SBUF tiles
        w = np.asarray(w)
        return np.ascontiguousarray(
            w.reshape(6, 128, w.shape[1]).transpose(1, 0, 2)
        )

    shared = {
        "wqk": shuf(W_s[:, : 2 * C].astype(BF16NP)),
        "wv": shuf(W_s[:, 2 * C :].astype(BF16NP)),
        "wp": shuf(np.asarray(W_proj, np.float32).astype(BF16NP)),
        "bqk": np.ascontiguousarray(b_s[: 2 * C].reshape(12, 128).T),
        "bv1": np.ascontiguousarray(b_s[2 * C :].reshape(1, C)).astype(
            np.float32
        ),
        "bp1": np.ascontiguousarray(
            np.asarray(b_proj, np.float32).reshape(1, C)
        ),
    }

    per_core = []
    for b in range(NCORES):
        xT = shuf(np.asarray(x[b], np.float32).T.astype(BF16NP))
        qm = np.ascontiguousarray(
            np.asarray(query_mask[b, 0, :, 0], np.float32).reshape(8, 128).T
        )
        mb = np.asarray(mask[b, 0])  # [T, T] bool
        blocks = [
            mb[qi * 128 : (qi + 1) * 128, qi * 128 : (qi + 1) * 128].T
            for qi in range(8)
        ]
        dm = np.stack(blocks, axis=1).astype(BF16NP)  # [128, 8, 128]
        per_core.append({"xT": xT, "qm": qm, "dm": dm, **shared})
    return per_core


def run_on_cores(inputs, trace=False, **kw):
    from concourse.bass_utils import run_bass_kernel_spmd

    nc = _get_nc()
    in_maps = prep_core_inputs(**inputs)
    res = run_bass_kernel_spmd(
        nc, in_maps, core_ids=list(range(NCORES)), trace=trace, **kw
    )
    out = np.stack([res.results[b]["y"] for b in range(NCORES)], axis=0)
    return out.astype(np.float32), res


def kernel(**inputs) -> np.ndarray:
    out, _ = run_on_cores(inputs, trace=False)
    return out
